# revision 1
# baseline (speedup 1.0000x reference)
"""GQA (softermax) Trainium2 kernel, tensor-parallel over kv-head groups.

Problem: x[1,2048,4096], 32 q-heads / 8 kv-heads, head_dim=128, base-2
softmax (softermax), fp32. Each of the 8 cores owns one kv-head group
(4 q-heads, 512 q dims, 128 kv dims) and computes a full partial
o-projection [2048,4096]; the host sums the 8 partials and adds o_b.

Per-core dataflow (all matmuls fp32 via float32r, 1 cycle/row):
  pass A: x -> xT (PE transpose) -> K^T,V^T ([d, seq]) -> V (natural)
  pass B: x -> xT again -> Q^T ([dq, seq])
  attn:   S^T[kb,q] = (K^T chunk).T @ Q^T ; P^T = exp(S^T * ln2/sqrt(128))
          sums over k: DVE chunk-adds + ones-matmul ; O^T[d,q] = V.T @ P^T
          O^T scaled by broadcasted 1/sums (K=1 matmul broadcast)
  oproj (fused per q-block): out[seq,e] = (O^T chunk).T @ o_wT chunk
"""

import math
from contextlib import ExitStack

import numpy as np

import concourse.bass as bass
from concourse import bacc
import concourse.mybir as mybir
import concourse.tile as tile
from concourse.bass_utils import run_bass_kernel_spmd
from concourse.masks import make_identity

E = 4096          # embed dim
S = 2048          # sequence
D = 128           # head dim
NHL = 4           # q heads per core
DQ = NHL * D      # 512 q dims per core
DKV = 128         # kv dims per core (1 kv head)
NCORES = 8

SB = 256          # seq block for projection passes
NSB = S // SB
QS = 256          # q block in attention
NQS = S // QS
NKT = S // 128    # 16 k chunks
NE = E // 128     # 32 embed chunks

F32 = mybir.dt.float32
F32R = mybir.dt.float32r
EXP_SCALE = math.log(2.0) / math.sqrt(D)

_CACHED_NC = None


def r(ap):
    return ap.bitcast(F32R)


def build_bass():
    nc = bacc.Bacc(None)

    x_d = nc.declare_dram_parameter("x", [S, E], F32, isOutput=False)
    qw_d = nc.declare_dram_parameter("qw", [DQ, E], F32, isOutput=False)
    qb_d = nc.declare_dram_parameter("qb", [DQ], F32, isOutput=False)
    kw_d = nc.declare_dram_parameter("kw", [DKV, E], F32, isOutput=False)
    kb_d = nc.declare_dram_parameter("kb", [DKV], F32, isOutput=False)
    vw_d = nc.declare_dram_parameter("vw", [DKV, E], F32, isOutput=False)
    vb_d = nc.declare_dram_parameter("vb", [DKV], F32, isOutput=False)
    ow_d = nc.declare_dram_parameter("ow", [E, DQ], F32, isOutput=False)
    out_d = nc.declare_dram_parameter("out", [S, E], F32, isOutput=True)

    Id = mybir.ActivationFunctionType.Identity
    Exp = mybir.ActivationFunctionType.Exp

    def copy_ps(i, dst, src):
        # alternate psum->sbuf copies between ACT and DVE
        if i % 2 == 0:
            nc.scalar.copy(dst, src)
        else:
            nc.vector.tensor_copy(dst, src)

    with tile.TileContext(nc) as tc, ExitStack() as es:
        consts = es.enter_context(tc.tile_pool(name="consts", bufs=1))
        persist = es.enter_context(tc.tile_pool(name="persist", bufs=1))

        # ---------------- constants ----------------
        ident = consts.tile([128, 128], F32)
        make_identity(nc, ident[:, :])
        ones_col = consts.tile([128, 1], F32)
        nc.gpsimd.memset(ones_col[:, :], 1.0)
        ones_row = consts.tile([1, 128], F32)
        nc.gpsimd.memset(ones_row[:, :], 1.0)

        qb_sb = consts.tile([128, NHL], F32)
        nc.sync.dma_start(qb_sb[:, :], qb_d[:].rearrange("(t p) -> p t", p=128))
        kb_sb = consts.tile([128, 1], F32)
        nc.sync.dma_start(kb_sb[:, :], kb_d[:].rearrange("(p o) -> p o", o=1))
        vb_sb = consts.tile([128, 1], F32)
        nc.sync.dma_start(vb_sb[:, :], vb_d[:].rearrange("(p o) -> p o", o=1))

        # ---------------- persistent tensors (48 KB/partition) ----------
        KT = persist.tile([128, S], F32R)            # K^T [d, seq]
        Vn = persist.tile([128, NKT, 128], F32R)     # V natural [seq, d] chunks
        QT = persist.tile([128, NHL, S], F32R)       # Q^T per head [d, seq]

        def load_transpose_x(xz, xtp, ps_tr, sb):
            xT = xtp.tile([128, NE, SB], F32R, tag="xT")
            for rt in range(SB // 128):
                xrow = xz.tile([128, E], F32, tag="xrow")
                row0 = sb * SB + rt * 128
                nc.sync.dma_start(xrow[:, :], x_d[row0:row0 + 128, :])
                for e in range(NE):
                    tp = ps_tr.tile([128, 128], F32, tag="tr")
                    nc.tensor.transpose(
                        tp[:, :], xrow[:, e * 128:(e + 1) * 128], ident[:, :])
                    copy_ps(e, xT[:, e, rt * 128:(rt + 1) * 128], tp[:, :])
            return xT

        with (
            tc.tile_pool(name="xz", bufs=3) as xz,
            tc.tile_pool(name="xtp", bufs=1) as xtp,
            tc.tile_pool(name="ps_tr", bufs=4, space="PSUM") as ps_tr,
            tc.tile_pool(name="ps_acc", bufs=2, space="PSUM") as ps_acc,
        ):
            with tc.tile_pool(name="wkv", bufs=1) as wkv:
                # ---- k/v weights: load + transpose ----
                kwT = wkv.tile([128, NE, 128], F32R, tag="kwT")
                vwT = wkv.tile([128, NE, 128], F32R, tag="vwT")
                VT = wkv.tile([128, S], F32, tag="VT")
                knat = xz.tile([128, E], F32, tag="xrow")
                nc.sync.dma_start(knat[:, :], kw_d[:, :])
                vnat = xz.tile([128, E], F32, tag="xrow")
                nc.sync.dma_start(vnat[:, :], vw_d[:, :])
                for e in range(NE):
                    tp = ps_tr.tile([128, 128], F32, tag="tr")
                    nc.tensor.transpose(
                        tp[:, :], knat[:, e * 128:(e + 1) * 128], ident[:, :])
                    copy_ps(e, kwT[:, e, :], tp[:, :])
                for e in range(NE):
                    tp = ps_tr.tile([128, 128], F32, tag="tr")
                    nc.tensor.transpose(
                        tp[:, :], vnat[:, e * 128:(e + 1) * 128], ident[:, :])
                    copy_ps(e + 1, vwT[:, e, :], tp[:, :])

                # ---- pass A: K^T, V^T, V natural ----
                for sb in range(NSB):
                    xT = load_transpose_x(xz, xtp, ps_tr, sb)
                    ssl = slice(sb * SB, (sb + 1) * SB)
                    ps_k = ps_acc.tile([128, SB], F32, tag="acc")
                    for e in range(NE):
                        nc.tensor.matmul(ps_k[:, :], kwT[:, e, :],
                                         xT[:, e, :],
                                         start=(e == 0), stop=(e == NE - 1))
                    nc.scalar.activation(KT[:, ssl], ps_k[:, :], Id,
                                         bias=kb_sb[:, 0:1])

                    ps_v = ps_acc.tile([128, SB], F32, tag="acc")
                    for e in range(NE):
                        nc.tensor.matmul(ps_v[:, :], vwT[:, e, :],
                                         xT[:, e, :],
                                         start=(e == 0), stop=(e == NE - 1))
                    nc.scalar.activation(VT[:, ssl], ps_v[:, :], Id,
                                         bias=vb_sb[:, 0:1])

                    for i in range(SB // 128):
                        t = sb * (SB // 128) + i
                        tp = ps_tr.tile([128, 128], F32, tag="tr")
                        nc.tensor.transpose(tp[:, :], VT[:, t * 128:(t + 1) * 128],
                                            ident[:, :])
                        copy_ps(i, Vn[:, t, :], tp[:, :])

            with tc.tile_pool(name="wq", bufs=1) as wq:
                # ---- q weights: load + transpose ----
                qwT = wq.tile([128, NE, DQ], F32R, tag="qwT")
                for t in range(NHL):
                    qnat = xz.tile([128, E], F32, tag="xrow")
                    nc.sync.dma_start(qnat[:, :], qw_d[t * 128:(t + 1) * 128, :])
                    for e in range(NE):
                        tp = ps_tr.tile([128, 128], F32, tag="tr")
                        nc.tensor.transpose(
                            tp[:, :], qnat[:, e * 128:(e + 1) * 128], ident[:, :])
                        copy_ps(e, qwT[:, e, t * 128:(t + 1) * 128], tp[:, :])

                # ---- pass B: Q^T ----
                for sb in range(NSB):
                    xT = load_transpose_x(xz, xtp, ps_tr, sb)
                    ssl = slice(sb * SB, (sb + 1) * SB)
                    for h in range(NHL):
                        ps_q = ps_acc.tile([128, SB], F32, tag="acc")
                        for e in range(NE):
                            nc.tensor.matmul(
                                ps_q[:, :],
                                qwT[:, e, h * 128:(h + 1) * 128],
                                xT[:, e, :],
                                start=(e == 0), stop=(e == NE - 1))
                        nc.scalar.activation(QT[:, h, ssl], ps_q[:, :], Id,
                                             bias=qb_sb[:, h:h + 1])

        with tc.tile_pool(name="wo", bufs=1) as wo:
            owT = wo.tile([128, NHL, E], F32R, tag="owT")
            with (
                tc.tile_pool(name="ost", bufs=2) as ost,
                tc.tile_pool(name="ps_tr2", bufs=4, space="PSUM") as ps_tr2,
            ):
                for m in range(NE):
                    onat = ost.tile([128, DQ], F32, tag="onat")
                    nc.sync.dma_start(onat[:, :], ow_d[m * 128:(m + 1) * 128, :])
                    for dh in range(NHL):
                        tp = ps_tr2.tile([128, 128], F32, tag="tr")
                        nc.tensor.transpose(
                            tp[:, :], onat[:, dh * 128:(dh + 1) * 128], ident[:, :])
                        copy_ps(m + dh, owT[:, dh, m * 128:(m + 1) * 128], tp[:, :])

            # ---------------- attention + fused o-projection ----------------
            with (
                tc.tile_pool(name="attn", bufs=2) as attn,
                tc.tile_pool(name="attn1", bufs=1) as attn1,
                tc.tile_pool(name="obp", bufs=3) as obp,
                tc.tile_pool(name="ps_s", bufs=2, space="PSUM") as ps_s,
                tc.tile_pool(name="ps_o", bufs=2, space="PSUM") as ps_o,
                tc.tile_pool(name="ps_sm", bufs=1, space="PSUM") as ps_sm,
                tc.tile_pool(name="ps_po", bufs=2, space="PSUM") as ps_po,
            ):
                for qi in range(NQS):
                    qsl = slice(qi * QS, (qi + 1) * QS)
                    OTb = attn.tile([128, NHL, QS], F32R, tag="OTb")
                    for h in range(NHL):
                        PT = attn.tile([128, NKT, QS], F32R, tag="PT")
                        for kt in range(NKT):
                            sps = ps_s.tile([128, QS], F32, tag="s")
                            nc.tensor.matmul(sps[:, :],
                                             KT[:, kt * 128:(kt + 1) * 128],
                                             QT[:, h, qsl],
                                             start=True, stop=True)
                            nc.scalar.activation(PT[:, kt, :], sps[:, :], Exp,
                                                 scale=EXP_SCALE)
                        ops = ps_o.tile([128, QS], F32, tag="o")
                        for kt in range(NKT):
                            nc.tensor.matmul(ops[:, :], Vn[:, kt, :],
                                             PT[:, kt, :],
                                             start=(kt == 0), stop=(kt == NKT - 1))
                        # denominator: sum over k of P^T
                        acc = attn1.tile([128, QS], F32, tag="pacc")
                        nc.vector.tensor_add(acc[:, :], PT[:, 0, :].bitcast(F32), PT[:, 1, :].bitcast(F32))
                        for kt in range(2, NKT):
                            nc.vector.tensor_add(acc[:, :], acc[:, :], PT[:, kt, :].bitcast(F32))
                        sums = ps_sm.tile([1, QS], F32, tag="sums")
                        nc.tensor.matmul(sums[:, :], ones_col[:, :], acc[:, :],
                                         start=True, stop=True)
                        recip = attn1.tile([1, QS], F32, tag="recip")
                        nc.vector.reciprocal(recip[:, :], sums[:, :])
                        bc = ps_sm.tile([128, QS], F32, tag="bc")
                        nc.tensor.matmul(bc[:, :], ones_row[:, :], recip[:, :],
                                         start=True, stop=True)
                        bcs = attn1.tile([128, QS], F32, tag="bcs")
                        nc.scalar.copy(bcs[:, :], bc[:, :])
                        nc.vector.tensor_mul(OTb[:, h, qsl.start - qi * QS:
                                                 qsl.stop - qi * QS],
                                             ops[:, :], bcs[:, :])

                    # fused o-projection for this q block
                    for sl in range(QS // 128):
                        st0 = qi * QS + sl * 128
                        for ec in range(E // 512):
                            po = ps_po.tile([128, 512], F32, tag="po")
                            for dh in range(NHL):
                                nc.tensor.matmul(
                                    po[:, :],
                                    OTb[:, dh, sl * 128:(sl + 1) * 128],
                                    owT[:, dh, ec * 512:(ec + 1) * 512],
                                    start=(dh == 0), stop=(dh == NHL - 1))
                            ob = obp.tile([128, 512], F32, tag="ob")
                            copy_ps(sl + ec, ob[:, :], po[:, :])
                            nc.sync.dma_start(
                                out_d[st0:st0 + 128, ec * 512:(ec + 1) * 512],
                                ob[:, :])

    nc.finalize()
    return nc


def kernel(x, q_w, q_b, k_w, k_b, v_w, v_b, o_w, o_b):
    global _CACHED_NC
    x2 = np.ascontiguousarray(np.asarray(x, np.float32).reshape(S, E))
    in_maps = []
    for c in range(NCORES):
        qsl = slice(c * DQ, (c + 1) * DQ)
        ksl = slice(c * DKV, (c + 1) * DKV)
        in_maps.append({
            "x": x2,
            "qw": np.ascontiguousarray(np.asarray(q_w, np.float32)[qsl]),
            "qb": np.ascontiguousarray(np.asarray(q_b, np.float32)[qsl]),
            "kw": np.ascontiguousarray(np.asarray(k_w, np.float32)[ksl]),
            "kb": np.ascontiguousarray(np.asarray(k_b, np.float32)[ksl]),
            "vw": np.ascontiguousarray(np.asarray(v_w, np.float32)[ksl]),
            "vb": np.ascontiguousarray(np.asarray(v_b, np.float32)[ksl]),
            "ow": np.ascontiguousarray(np.asarray(o_w, np.float32)[:, qsl]),
        })
    if _CACHED_NC is None:
        _CACHED_NC = build_bass()
    res = run_bass_kernel_spmd(_CACHED_NC, in_maps, list(range(NCORES)))
    out = np.zeros((S, E), np.float64)
    for i in range(NCORES):
        out += res.results[i]["out"].astype(np.float64)
    out += np.asarray(o_b, np.float64)
    return out.astype(np.float32).reshape(1, S, E)



# revision 2
# speedup vs baseline: 1.8239x; 1.8239x over previous
"""GQA (softermax) Trainium2 kernel, tensor-parallel over kv-head groups.

Problem: x[1,2048,4096], 32 q-heads / 8 kv-heads, head_dim=128, base-2
softmax (softermax), fp32 reference. Each of the 8 cores owns one kv-head
group (4 q-heads, 512 q dims, 128 kv dims) and computes a partial
o-projection [2048,4096]; the host sums the 8 partials and adds o_b.

v2 (bf16): all matmuls in bf16 (1 cyc/row, FWL weight loads). The host
pre-transposes x and the weight matrices so no on-chip transposes of x or
weights are needed (only 16 small V transposes). Softmax denominators are
computed on the PE with an all-ones [128,128] stationary operand, which
both reduces over k-chunk partitions AND broadcasts Z to all 128
partitions in one accumulation group; 1/Z via reciprocal_approx_fast.

Per-core dataflow:
  proj:  xT[e,s] (DMA, host-transposed) ; K^T,V^T,Q^T = W^T.T @ xT
         V natural via 16 PE transposes of V^T
  attn:  S^T[k,q] = KT_chunk.T @ QT ; P^T = exp(S^T * ln2/sqrt(128)) [ACT]
         O^T[d,q] = sum_k Vn_chunk.T @ P^T_chunk   (PSUM accum)
         Zb[*,q]  = sum_k ones128.T @ P^T_chunk    (PSUM accum, all rows = Z)
         OTb = O^T * recip_approx(Zb)              [DVE]
  oproj: out[s,e] = sum_h OTb_h_chunk.T @ owT_h    (partial; host sums cores)
"""

import math
from contextlib import ExitStack

import numpy as np
import ml_dtypes

import concourse.bass as bass
from concourse import bacc
import concourse.mybir as mybir
import concourse.tile as tile
from concourse.bass_utils import run_bass_kernel_spmd
from concourse.masks import make_identity

E = 4096          # embed dim
S = 2048          # sequence
D = 128           # head dim
NHL = 4           # q heads per core
DQ = NHL * D      # 512 q dims per core
DKV = 128         # kv dims per core (1 kv head)
NCORES = 8
NE = E // 128     # 32 embed chunks

SB = 512          # seq block for projection pass
NSB = S // SB
QS = 512          # q block in attention
NQS = S // QS
NKT = S // 128    # 16 k chunks

F32 = mybir.dt.float32
BF = mybir.dt.bfloat16
BF_NP = ml_dtypes.bfloat16
EXP_SCALE = math.log(2.0) / math.sqrt(D)

_CACHED_NC = None


def build_bass():
    nc = bacc.Bacc(None)

    xt_d = nc.declare_dram_parameter("xt", [E, S], BF, isOutput=False)
    qwt_d = nc.declare_dram_parameter("qwt", [E, DQ], BF, isOutput=False)
    qb_d = nc.declare_dram_parameter("qb", [DQ], F32, isOutput=False)
    kwt_d = nc.declare_dram_parameter("kwt", [E, DKV], BF, isOutput=False)
    kb_d = nc.declare_dram_parameter("kb", [DKV], F32, isOutput=False)
    vwt_d = nc.declare_dram_parameter("vwt", [E, DKV], BF, isOutput=False)
    vb_d = nc.declare_dram_parameter("vb", [DKV], F32, isOutput=False)
    owt_d = nc.declare_dram_parameter("owt", [DQ, E], BF, isOutput=False)
    out_d = nc.declare_dram_parameter("out", [S, E], BF, isOutput=True)

    Id = mybir.ActivationFunctionType.Identity
    Exp = mybir.ActivationFunctionType.Exp

    def copy_ps(i, dst, src):
        # alternate psum->sbuf copies between ACT and DVE
        if i % 2 == 0:
            nc.scalar.copy(dst, src)
        else:
            nc.vector.tensor_copy(dst, src)

    with tile.TileContext(nc) as tc, ExitStack() as es:
        consts = es.enter_context(tc.tile_pool(name="consts", bufs=1))
        persist = es.enter_context(tc.tile_pool(name="persist", bufs=1))

        # ---------------- constants ----------------
        ident = consts.tile([128, 128], BF)
        make_identity(nc, ident[:, :])
        ones128 = consts.tile([128, 128], BF)
        nc.gpsimd.memset(ones128[:, :], 1.0)

        qb_sb = consts.tile([128, NHL], F32)
        nc.sync.dma_start(qb_sb[:, :], qb_d[:].rearrange("(t p) -> p t", p=128))
        kb_sb = consts.tile([128, 1], F32)
        nc.sync.dma_start(kb_sb[:, :], kb_d[:].rearrange("(p o) -> p o", o=1))
        vb_sb = consts.tile([128, 1], F32)
        nc.sync.dma_start(vb_sb[:, :], vb_d[:].rearrange("(p o) -> p o", o=1))

        # ---------------- persistent tensors (24 KB/partition) ----------
        KT = persist.tile([128, S], BF)              # K^T [d, seq]
        QT = persist.tile([128, NHL, S], BF)         # Q^T per head [d, seq]
        Vn = persist.tile([128, NKT, 128], BF)       # V natural [seq, d] chunks

        # ================= phase 1: projections =================
        with (
            tc.tile_pool(name="xz", bufs=2) as xz,
            tc.tile_pool(name="wproj", bufs=1) as wproj,
            tc.tile_pool(name="vtmp", bufs=2) as vtmp,
            tc.tile_pool(name="ps_acc", bufs=3, space="PSUM") as ps_acc,
            tc.tile_pool(name="ps_tr", bufs=4, space="PSUM") as ps_tr,
        ):
            kwT = wproj.tile([128, NE, DKV], BF, tag="kwT")
            nc.sync.dma_start(kwT[:, :, :],
                              kwt_d[:, :].rearrange("(g p) d -> p g d", p=128))
            vwT = wproj.tile([128, NE, DKV], BF, tag="vwT")
            nc.sync.dma_start(vwT[:, :, :],
                              vwt_d[:, :].rearrange("(g p) d -> p g d", p=128))
            qwT = wproj.tile([128, NE, DQ], BF, tag="qwT")
            nc.sync.dma_start(qwT[:, :, :],
                              qwt_d[:, :].rearrange("(g p) d -> p g d", p=128))

            for sb in range(NSB):
                ssl = slice(sb * SB, (sb + 1) * SB)
                xTb = xz.tile([128, NE, SB], BF, tag="x")
                nc.sync.dma_start(
                    xTb[:, :, :],
                    xt_d[:, ssl].rearrange("(g p) s -> p g s", p=128))

                ps_k = ps_acc.tile([128, SB], F32, tag="acc")
                for e in range(NE):
                    nc.tensor.matmul(ps_k[:, :], kwT[:, e, :], xTb[:, e, :],
                                     start=(e == 0), stop=(e == NE - 1))
                nc.scalar.activation(KT[:, ssl], ps_k[:, :], Id,
                                     bias=kb_sb[:, 0:1])

                ps_v = ps_acc.tile([128, SB], F32, tag="acc")
                for e in range(NE):
                    nc.tensor.matmul(ps_v[:, :], vwT[:, e, :], xTb[:, e, :],
                                     start=(e == 0), stop=(e == NE - 1))
                VTb = vtmp.tile([128, SB], BF, tag="vt")
                nc.scalar.activation(VTb[:, :], ps_v[:, :], Id,
                                     bias=vb_sb[:, 0:1])
                for i in range(SB // 128):
                    tp = ps_tr.tile([128, 128], BF, tag="tr")
                    nc.tensor.transpose(tp[:, :], VTb[:, i * 128:(i + 1) * 128],
                                        ident[:, :])
                    nc.vector.tensor_copy(Vn[:, sb * (SB // 128) + i, :],
                                          tp[:, :])

                for h in range(NHL):
                    ps_q = ps_acc.tile([128, SB], F32, tag="acc")
                    for e in range(NE):
                        nc.tensor.matmul(ps_q[:, :],
                                         qwT[:, e, h * 128:(h + 1) * 128],
                                         xTb[:, e, :],
                                         start=(e == 0), stop=(e == NE - 1))
                    nc.scalar.activation(QT[:, h, ssl], ps_q[:, :], Id,
                                         bias=qb_sb[:, h:h + 1])

        # ================= phase 2: attention + fused o-proj =================
        with (
            tc.tile_pool(name="wo", bufs=1) as wo,
            tc.tile_pool(name="attn", bufs=2) as attn,
            tc.tile_pool(name="obp", bufs=3) as obp,
            tc.tile_pool(name="ps_s", bufs=2, space="PSUM") as ps_s,
            tc.tile_pool(name="ps_o", bufs=2, space="PSUM") as ps_o,
            tc.tile_pool(name="ps_z", bufs=2, space="PSUM") as ps_z,
            tc.tile_pool(name="ps_po", bufs=2, space="PSUM") as ps_po,
        ):
            owT = wo.tile([128, NHL, E], BF, tag="owT")
            nc.sync.dma_start(owT[:, :, :],
                              owt_d[:, :].rearrange("(t p) e -> p t e", p=128))

            for qi in range(NQS):
                qsl = slice(qi * QS, (qi + 1) * QS)
                OTb = attn.tile([128, NHL, QS], BF, tag="OTb")
                for h in range(NHL):
                    PT = attn.tile([128, NKT, QS], BF, tag="PT")
                    for kt in range(NKT):
                        sps = ps_s.tile([128, QS], F32, tag="s")
                        nc.tensor.matmul(sps[:, :],
                                         KT[:, kt * 128:(kt + 1) * 128],
                                         QT[:, h, qsl],
                                         start=True, stop=True)
                        nc.scalar.activation(PT[:, kt, :], sps[:, :], Exp,
                                             scale=EXP_SCALE)
                    ops = ps_o.tile([128, QS], F32, tag="o")
                    for kt in range(NKT):
                        nc.tensor.matmul(ops[:, :], Vn[:, kt, :], PT[:, kt, :],
                                         start=(kt == 0), stop=(kt == NKT - 1))
                    # Z broadcast to all 128 partitions via all-ones stationary
                    zps = ps_z.tile([128, QS], F32, tag="z")
                    for kt in range(NKT):
                        nc.tensor.matmul(zps[:, :], ones128[:, :], PT[:, kt, :],
                                         start=(kt == 0), stop=(kt == NKT - 1))
                    bcs = attn.tile([128, QS], F32, tag="bcs")
                    nc.vector.reciprocal_approx_fast(bcs[:, :], zps[:, :])
                    nc.vector.tensor_mul(OTb[:, h, :], ops[:, :], bcs[:, :])

                # fused o-projection for this q block
                for sl in range(QS // 128):
                    st0 = qi * QS + sl * 128
                    for ec in range(E // 512):
                        po = ps_po.tile([128, 512], F32, tag="po")
                        for dh in range(NHL):
                            nc.tensor.matmul(
                                po[:, :],
                                OTb[:, dh, sl * 128:(sl + 1) * 128],
                                owT[:, dh, ec * 512:(ec + 1) * 512],
                                start=(dh == 0), stop=(dh == NHL - 1))
                        ob = obp.tile([128, 512], BF, tag="ob")
                        copy_ps(sl + ec, ob[:, :], po[:, :])
                        nc.sync.dma_start(
                            out_d[st0:st0 + 128, ec * 512:(ec + 1) * 512],
                            ob[:, :])

    nc.finalize()
    return nc


def make_in_maps(x, q_w, q_b, k_w, k_b, v_w, v_b, o_w):
    x2 = np.asarray(x, np.float32).reshape(S, E)
    xt = np.ascontiguousarray(x2.T).astype(BF_NP)
    q_w = np.asarray(q_w, np.float32)
    k_w = np.asarray(k_w, np.float32)
    v_w = np.asarray(v_w, np.float32)
    o_w = np.asarray(o_w, np.float32)
    in_maps = []
    for c in range(NCORES):
        qsl = slice(c * DQ, (c + 1) * DQ)
        ksl = slice(c * DKV, (c + 1) * DKV)
        in_maps.append({
            "xt": xt,
            "qwt": np.ascontiguousarray(q_w[qsl].T).astype(BF_NP),
            "qb": np.ascontiguousarray(np.asarray(q_b, np.float32)[qsl]),
            "kwt": np.ascontiguousarray(k_w[ksl].T).astype(BF_NP),
            "kb": np.ascontiguousarray(np.asarray(k_b, np.float32)[ksl]),
            "vwt": np.ascontiguousarray(v_w[ksl].T).astype(BF_NP),
            "vb": np.ascontiguousarray(np.asarray(v_b, np.float32)[ksl]),
            "owt": np.ascontiguousarray(o_w[:, qsl].T).astype(BF_NP),
        })
    return in_maps


def kernel(x, q_w, q_b, k_w, k_b, v_w, v_b, o_w, o_b):
    global _CACHED_NC
    in_maps = make_in_maps(x, q_w, q_b, k_w, k_b, v_w, v_b, o_w)
    if _CACHED_NC is None:
        _CACHED_NC = build_bass()
    res = run_bass_kernel_spmd(_CACHED_NC, in_maps, list(range(NCORES)))
    out = np.zeros((S, E), np.float64)
    for i in range(NCORES):
        out += res.results[i]["out"].astype(np.float64)
    out += np.asarray(o_b, np.float64)
    return out.astype(np.float32).reshape(1, S, E)


# revision 4
# speedup vs baseline: 2.1059x; 1.1546x over previous
"""GQA (softermax) Trainium2 kernel, tensor-parallel over kv-head groups.

Problem: x[1,2048,4096], 32 q-heads / 8 kv-heads, head_dim=128, base-2
softmax (softermax), fp32 reference. Each of the 8 cores owns one kv-head
group (4 q-heads, 512 q dims, 128 kv dims) and computes a partial
o-projection [2048,4096]; the host sums the 8 partials and adds o_b.

v3 (bf16, SBUF-layout DMA): all matmuls bf16 (1 cyc/row, FWL weight
loads). The host pre-transposes/pre-chunks x and weights into the exact
SBUF tile layouts so every DMA descriptor is an 8-32KB contiguous run
per partition (the naive [E,S] layout gave ~1KB descriptors and only
~5 GB/s per DMA engine). Softmax denominators: DVE bf16 chunk-add chain
(2x packed mode) + one all-ones [128,128] stationary matmul that both
partition-reduces and broadcasts Z; 1/Z via reciprocal_approx_fast.
Exp runs on paired score chunks (FD=1024 spanning 2 PSUM banks) to
amortize ACT per-call overhead.

Per-core dataflow:
  proj:  xT[e,s] (DMA, host-chunked) ; K^T,V^T,Q^T = W^T.T @ xT
         V natural via 16 PE transposes of V^T
  attn:  S^T[k,q] = KT_chunk.T @ QT (pairs of chunks into one 2-bank
         PSUM tile); P^T = exp(S^T * ln2/sqrt(128)) [ACT, FD=1024]
         O^T[d,q] = sum_k Vn_chunk.T @ P^T_chunk   (PSUM accum)
         acc = bf16 chunk-sum of P^T [DVE]; Zb = ones128.T @ acc (PE)
         OTb = O^T * recip_approx(Zb)              [DVE]
  oproj: out[s,e] = sum_h OTb_h_chunk.T @ owT_h    (partial; host sums)
"""

import math
from contextlib import ExitStack

import numpy as np
import ml_dtypes

import concourse.bass as bass
from concourse import bacc
import concourse.mybir as mybir
import concourse.tile as tile
from concourse.bass_utils import run_bass_kernel_spmd
from concourse.masks import make_identity

E = 4096          # embed dim
S = 2048          # sequence
D = 128           # head dim
NHL = 4           # q heads per core
DQ = NHL * D      # 512 q dims per core
DKV = 128         # kv dims per core (1 kv head)
NCORES = 8
NE = E // 128     # 32 embed chunks

SB = 512          # seq block for projection pass
NSB = S // SB
QS = 512          # q block in attention
NQS = S // QS
NKT = S // 128    # 16 k chunks
NOB = S // 128    # 16 output row blocks
NOE = E // 512    # 8 output col blocks

F32 = mybir.dt.float32
BF = mybir.dt.bfloat16
BF_NP = ml_dtypes.bfloat16
EXP_SCALE = math.log(2.0) / math.sqrt(D)

_CACHED_NC = None


def build_bass():
    nc = bacc.Bacc(None)

    # all inputs pre-chunked on host into SBUF tile layout [p, chunk, cols]
    xt_d = nc.declare_dram_parameter("xt", [NSB, 128, NE, SB], BF,
                                     isOutput=False)
    qwt_d = nc.declare_dram_parameter("qwt", [128, NE, DQ], BF, isOutput=False)
    qb_d = nc.declare_dram_parameter("qb", [DQ], F32, isOutput=False)
    kwt_d = nc.declare_dram_parameter("kwt", [128, NE, DKV], BF, isOutput=False)
    kb_d = nc.declare_dram_parameter("kb", [DKV], F32, isOutput=False)
    vwt_d = nc.declare_dram_parameter("vwt", [128, NE, DKV], BF, isOutput=False)
    vb_d = nc.declare_dram_parameter("vb", [DKV], F32, isOutput=False)
    owt_d = nc.declare_dram_parameter("owt", [128, NHL, E], BF, isOutput=False)
    out_d = nc.declare_dram_parameter("out", [NOB, NOE, 128, 512], BF,
                                      isOutput=True)

    Id = mybir.ActivationFunctionType.Identity
    Exp = mybir.ActivationFunctionType.Exp

    def copy_ps(i, dst, src):
        # alternate psum->sbuf copies between ACT and DVE
        if i % 2 == 0:
            nc.scalar.copy(dst, src)
        else:
            nc.vector.tensor_copy(dst, src)

    with tile.TileContext(nc) as tc, ExitStack() as es:
        consts = es.enter_context(tc.tile_pool(name="consts", bufs=1))
        persist = es.enter_context(tc.tile_pool(name="persist", bufs=1))

        # ---------------- constants ----------------
        ident = consts.tile([128, 128], BF)
        make_identity(nc, ident[:, :])
        ones128 = consts.tile([128, 128], BF)
        nc.gpsimd.memset(ones128[:, :], 1.0)

        qb_sb = consts.tile([128, NHL], F32)
        nc.sync.dma_start(qb_sb[:, :], qb_d[:].rearrange("(t p) -> p t", p=128))
        kb_sb = consts.tile([128, 1], F32)
        nc.sync.dma_start(kb_sb[:, :], kb_d[:].rearrange("(p o) -> p o", o=1))
        vb_sb = consts.tile([128, 1], F32)
        nc.sync.dma_start(vb_sb[:, :], vb_d[:].rearrange("(p o) -> p o", o=1))

        # warm the ACT exp table during the initial DMA wait
        warm = consts.tile([128, 1], F32)
        nc.scalar.activation(warm[:, :], kb_sb[:, 0:1], Exp, scale=1.0)

        # ---------------- persistent tensors (24 KB/partition) ----------
        KT = persist.tile([128, S], BF)              # K^T [d, seq]
        QT = persist.tile([128, NHL, S], BF)         # Q^T per head [d, seq]
        Vn = persist.tile([128, NKT, 128], BF)       # V natural [seq, d] chunks

        # ================= phase 1: projections =================
        with (
            tc.tile_pool(name="xz", bufs=2) as xz,
            tc.tile_pool(name="wproj", bufs=1) as wproj,
            tc.tile_pool(name="vtmp", bufs=2) as vtmp,
            tc.tile_pool(name="ps_acc", bufs=3, space="PSUM") as ps_acc,
            tc.tile_pool(name="ps_tr", bufs=4, space="PSUM") as ps_tr,
        ):
            kwT = wproj.tile([128, NE, DKV], BF, tag="kwT")
            nc.sync.dma_start(kwT[:, :, :], kwt_d[:, :, :])
            vwT = wproj.tile([128, NE, DKV], BF, tag="vwT")
            nc.sync.dma_start(vwT[:, :, :], vwt_d[:, :, :])
            qwT = wproj.tile([128, NE, DQ], BF, tag="qwT")

            for sb in range(NSB):
                ssl = slice(sb * SB, (sb + 1) * SB)
                xTb = xz.tile([128, NE, SB], BF, tag="x")
                nc.sync.dma_start(xTb[:, :, :], xt_d[sb, :, :, :])
                if sb == 0:
                    # queue the big q weight load behind x block 0
                    nc.sync.dma_start(qwT[:, :, :], qwt_d[:, :, :])

                ps_k = ps_acc.tile([128, SB], F32, tag="acc")
                for e in range(NE):
                    nc.tensor.matmul(ps_k[:, :], kwT[:, e, :], xTb[:, e, :],
                                     start=(e == 0), stop=(e == NE - 1))
                nc.scalar.activation(KT[:, ssl], ps_k[:, :], Id,
                                     bias=kb_sb[:, 0:1])

                ps_v = ps_acc.tile([128, SB], F32, tag="acc")
                for e in range(NE):
                    nc.tensor.matmul(ps_v[:, :], vwT[:, e, :], xTb[:, e, :],
                                     start=(e == 0), stop=(e == NE - 1))
                VTb = vtmp.tile([128, SB], BF, tag="vt")
                nc.scalar.activation(VTb[:, :], ps_v[:, :], Id,
                                     bias=vb_sb[:, 0:1])
                for i in range(SB // 128):
                    tp = ps_tr.tile([128, 128], BF, tag="tr")
                    nc.tensor.transpose(tp[:, :], VTb[:, i * 128:(i + 1) * 128],
                                        ident[:, :])
                    nc.vector.tensor_copy(Vn[:, sb * (SB // 128) + i, :],
                                          tp[:, :])

                for h in range(NHL):
                    ps_q = ps_acc.tile([128, SB], F32, tag="acc")
                    for e in range(NE):
                        nc.tensor.matmul(ps_q[:, :],
                                         qwT[:, e, h * 128:(h + 1) * 128],
                                         xTb[:, e, :],
                                         start=(e == 0), stop=(e == NE - 1))
                    nc.scalar.activation(QT[:, h, ssl], ps_q[:, :], Id,
                                         bias=qb_sb[:, h:h + 1])

        # ================= phase 2: attention + fused o-proj =================
        with (
            tc.tile_pool(name="wo", bufs=1) as wo,
            tc.tile_pool(name="attn", bufs=2) as attn,
            tc.tile_pool(name="obp", bufs=3) as obp,
            tc.tile_pool(name="ps_s", bufs=2, space="PSUM") as ps_s,
            tc.tile_pool(name="ps_o", bufs=2, space="PSUM") as ps_o,
            tc.tile_pool(name="ps_po", bufs=2, space="PSUM") as ps_po,
        ):
            owT = wo.tile([128, NHL, E], BF, tag="owT")
            nc.sync.dma_start(owT[:, :, :], owt_d[:, :, :])

            for qi in range(NQS):
                qsl = slice(qi * QS, (qi + 1) * QS)
                OTb = attn.tile([128, NHL, QS], BF, tag="OTb")
                for h in range(NHL):
                    PT = attn.tile([128, NKT, QS], BF, tag="PT")
                    for kp in range(NKT // 2):
                        sps = ps_s.tile([128, 2, QS], F32, tag="s")
                        for j in range(2):
                            kt = kp * 2 + j
                            nc.tensor.matmul(sps[:, j, :],
                                             KT[:, kt * 128:(kt + 1) * 128],
                                             QT[:, h, qsl],
                                             start=True, stop=True)
                        # exp over both chunks in one ACT call (FD=1024)
                        nc.scalar.activation(PT[:, kp * 2:kp * 2 + 2, :],
                                             sps[:, :, :], Exp,
                                             scale=EXP_SCALE)
                    ops = ps_o.tile([128, QS], F32, tag="o")
                    for kt in range(NKT):
                        nc.tensor.matmul(ops[:, :], Vn[:, kt, :], PT[:, kt, :],
                                         start=(kt == 0), stop=(kt == NKT - 1))
                    # Z: bf16 chunk-sum on DVE, then one all-ones matmul that
                    # partition-reduces AND broadcasts to all 128 rows
                    acc = attn.tile([128, QS], BF, tag="acc")
                    nc.vector.tensor_add(acc[:, :], PT[:, 0, :], PT[:, 1, :])
                    for kt in range(2, NKT):
                        nc.vector.tensor_add(acc[:, :], acc[:, :], PT[:, kt, :])
                    zps = ps_po.tile([128, QS], F32, tag="po")
                    nc.tensor.matmul(zps[:, :], ones128[:, :], acc[:, :],
                                     start=True, stop=True)
                    bcs = attn.tile([128, QS], F32, tag="bcs")
                    nc.vector.reciprocal_approx_fast(bcs[:, :], zps[:, :])
                    nc.vector.tensor_mul(OTb[:, h, :], ops[:, :], bcs[:, :])

                # fused o-projection for this q block
                for sl in range(QS // 128):
                    blk = qi * (QS // 128) + sl
                    for ec in range(NOE):
                        po = ps_po.tile([128, 512], F32, tag="po")
                        for dh in range(NHL):
                            nc.tensor.matmul(
                                po[:, :],
                                OTb[:, dh, sl * 128:(sl + 1) * 128],
                                owT[:, dh, ec * 512:(ec + 1) * 512],
                                start=(dh == 0), stop=(dh == NHL - 1))
                        ob = obp.tile([128, 512], BF, tag="ob")
                        copy_ps(sl + ec, ob[:, :], po[:, :])
                        nc.sync.dma_start(out_d[blk, ec, :, :], ob[:, :])

    nc.finalize()
    return nc


def make_in_maps(x, q_w, q_b, k_w, k_b, v_w, v_b, o_w):
    x2 = np.asarray(x, np.float32).reshape(S, E)
    # xt[sb, p, g, sl] = x[sb*SB+sl, g*128+p]
    xt = np.ascontiguousarray(
        x2.T.reshape(NE, 128, NSB, SB).transpose(2, 1, 0, 3)).astype(BF_NP)
    q_w = np.asarray(q_w, np.float32)
    k_w = np.asarray(k_w, np.float32)
    v_w = np.asarray(v_w, np.float32)
    o_w = np.asarray(o_w, np.float32)
    in_maps = []
    for c in range(NCORES):
        qsl = slice(c * DQ, (c + 1) * DQ)
        ksl = slice(c * DKV, (c + 1) * DKV)
        # w^T [E, dout] chunked to [p, g, dout]
        qwt = q_w[qsl].T.reshape(NE, 128, DQ).transpose(1, 0, 2)
        kwt = k_w[ksl].T.reshape(NE, 128, DKV).transpose(1, 0, 2)
        vwt = v_w[ksl].T.reshape(NE, 128, DKV).transpose(1, 0, 2)
        # o_w slice^T [DQ, E] chunked to [p, h, E]
        owt = o_w[:, qsl].T.reshape(NHL, 128, E).transpose(1, 0, 2)
        in_maps.append({
            "xt": xt,
            "qwt": np.ascontiguousarray(qwt).astype(BF_NP),
            "qb": np.ascontiguousarray(np.asarray(q_b, np.float32)[qsl]),
            "kwt": np.ascontiguousarray(kwt).astype(BF_NP),
            "kb": np.ascontiguousarray(np.asarray(k_b, np.float32)[ksl]),
            "vwt": np.ascontiguousarray(vwt).astype(BF_NP),
            "vb": np.ascontiguousarray(np.asarray(v_b, np.float32)[ksl]),
            "owt": np.ascontiguousarray(owt).astype(BF_NP),
        })
    return in_maps


def kernel(x, q_w, q_b, k_w, k_b, v_w, v_b, o_w, o_b):
    global _CACHED_NC
    in_maps = make_in_maps(x, q_w, q_b, k_w, k_b, v_w, v_b, o_w)
    if _CACHED_NC is None:
        _CACHED_NC = build_bass()
    res = run_bass_kernel_spmd(_CACHED_NC, in_maps, list(range(NCORES)))
    out = np.zeros((S, E), np.float64)
    for i in range(NCORES):
        o = res.results[i]["out"].astype(np.float32)
        out += o.transpose(0, 2, 1, 3).reshape(S, E).astype(np.float64)
    out += np.asarray(o_b, np.float64)
    return out.astype(np.float32).reshape(1, S, E)
